# revision 9
# baseline (speedup 1.0000x reference)
"""CTRNN (Dale-constrained leaky RNN) Trainium2 kernel, v3.

Math (per reference):
    Weff    = |Wahh| * mask
    xin_t   = x_t @ Wahx.T + bah
    ah_{t+1} = 0.9*ah_t + 0.1*(retanh(ah_t) @ Weff.T + xin_t)
    hs[t]   = retanh(ah_{t+1});   y = hs @ Wyh.T + by
    retanh(a) = max(tanh(a), 0) = tanh(max(a, 0))

Strategy: data-parallel over batch (B=64 -> 8 per NeuronCore), neuron-major
state [128, chunk, batch], recurrent weight stationary on the PE (bf16,
fp32 PSUM accumulate). The per-step PE stream is LDWEIGHTS-bandwidth bound
(~26 ns per 128x128 weight tile, clock-independent), so the step period is
set by the dependency cycle, not PE throughput.

v3 structure:
- The input drive AND bias are accumulated directly into PSUM: per block,
  bank mi is opened with a rank-1 bias matmul (0.1*bah chunk x ones) plus
  an input matmul (0.1*Wahx chunk @ x_t), and the 50 steps' recurrent
  matmuls accumulate on top at element offsets t*BL. The state update is a
  single fused DVE op: ah = 0.9*ah + psum. No xin HBM round trip, no
  per-block DMA, no separate u op.
- Parity-alternating skewed schedule: at step t the halves H[t%2]/H[1-t%2]
  update first/second. Quad order (first, k-early)(second, k-early)
  (first, k-late -> update first)(second, k-late -> update second) makes
  the binding dependency cycle one 16-matmul quad + the update chain,
  instead of 48 matmuls + chain.
- The y readout runs per block out of the PSUM bank tails (offsets
  400..512 of banks 4..7), overlapped at block boundaries.
"""

import numpy as np
import ml_dtypes

import concourse.bass as bass
import concourse.bacc as bacc
import concourse.mybir as mybir
from concourse.tile import TileContext
from concourse.bass_utils import run_bass_kernel_spmd
from concourse.alu_op_type import AluOpType

F32 = mybir.dt.float32
BF16 = mybir.dt.bfloat16
AF = mybir.ActivationFunctionType

B, T, NI, N, NO = 64, 1000, 128, 1024, 64
NCORES = 8
BL = B // NCORES            # batches per core
MC = N // 128               # m-chunks (output neuron chunks)
KC = N // 128               # k-chunks (contraction chunks)
U = 50                      # timesteps per block
NB = T // U                 # blocks
DT = 0.1                    # dt/tau
DECAY = 1.0 - DT
MH = MC // 2                # m-chunks per half
KH = KC // 2                # k-chunks per half
# y readout slot ranges per PSUM bank tail (112 fp32 capacity each)
YSPLIT = [(0, 14), (14, 28), (28, 42), (42, 50)]

TRACE = False               # set by test harness for profiling
LAST_RESULTS = None         # BassKernelResults of the last run


def _bcast_ap(t, shape_counts, steps):
    """Build an AP on tile t with explicit [step, count] pairs (after the
    partition dim, which is taken from t)."""
    ap = [t.ap[0]] + [[s, c] for s, c in zip(steps, shape_counts)]
    return bass.AP(tensor=t.tensor, offset=t.offset, ap=ap)


def _order_dep(after, before):
    """Order-only scheduling edge: `after` must be queued after `before`.
    sync=False: pure queue-order hint, no semaphore emitted."""
    from concourse.tile_rust import add_dep_helper
    a = getattr(after, "ins", after)
    b = getattr(before, "ins", before)
    add_dep_helper(a, b, sync=False, reason="manual queue order")


def _build_nc():
    nc = bacc.Bacc("TRN2", target_bir_lowering=False)

    x_d = nc.dram_tensor("x", [BL * T, NI], BF16, kind="ExternalInput")
    wq_d = nc.dram_tensor("wq", [128, KC, MC, 128], BF16, kind="ExternalInput")
    wx_d = nc.dram_tensor("wx", [NI, MC, 128], BF16, kind="ExternalInput")
    wy_d = nc.dram_tensor("wy", [128, KC, NO], BF16, kind="ExternalInput")
    bahq_d = nc.dram_tensor("bahq", [1, MC, 128], BF16, kind="ExternalInput")
    ones_d = nc.dram_tensor("ones", [1, U * BL], BF16, kind="ExternalInput")
    ah0_d = nc.dram_tensor("ah0", [128, MC], F32, kind="ExternalInput")
    by_d = nc.dram_tensor("by", [NO, 1], F32, kind="ExternalInput")
    y_d = nc.dram_tensor("y", [NO, T, BL], F32, kind="ExternalOutput")

    with TileContext(nc) as tc:
        with tc.tile_pool(name="consts", bufs=1) as consts, \
             tc.tile_pool(name="bigps", bufs=1, space="PSUM") as bigps:
            wsta = consts.tile([128, KC, MC, 128], BF16)
            nc.sync.dma_start(wsta, wq_d[:])
            wx = consts.tile([NI, MC, 128], BF16)
            nc.sync.dma_start(wx, wx_d[:])
            wy = consts.tile([128, KC, NO], BF16)
            nc.sync.dma_start(wy, wy_d[:])
            bahq = consts.tile([1, MC, 128], BF16)
            nc.sync.dma_start(bahq, bahq_d[:])
            ones = consts.tile([1, U * BL], BF16)
            nc.sync.dma_start(ones, ones_d[:])
            ah0T = consts.tile([128, MC], F32)
            nc.sync.dma_start(ah0T, ah0_d[:])
            byv = consts.tile([NO, 1], F32)
            nc.sync.dma_start(byv, by_d[:])

            xT = consts.tile([NI, BL * T], BF16)        # x transposed, bf16
            yb = consts.tile([NO, NB, U, BL], F32)      # y accumulator
            ah = consts.tile([128, MC, BL], F32)        # recurrent state
            ring = consts.tile([128, U, KC, BL], BF16)  # h ring (slot, chunk, b)

            big = bigps.tile([128, MC, 512], F32)       # all 8 PSUM banks

            # ---- Phase 1: transpose x -> xT via DMA xbar transpose ----
            nc.sync.dma_start_transpose(xT, x_d[:])
            # x as [p, block, u, b] for the per-block xin matmuls
            xTv = xT.rearrange("p (b nb u) -> p nb u b", nb=NB, u=U)

            # ---- Init: ah = broadcast(ah0), ring[U-1] = retanh(ah) ----
            with tc.tile_pool(name="initp", bufs=1) as initp:
                ah0b = _bcast_ap(ah0T, [MC, BL], [1, 0])
                nc.vector.tensor_copy(ah, ah0b)
                r0t = initp.tile([128, MC, BL], F32)
                nc.vector.tensor_scalar_max(r0t, ah, 0.0)
                nc.scalar.activation(ring[:, U - 1], r0t, AF.Tanh)

            # ---- The scan ----
            with tc.tile_pool(name="rrpool", bufs=4) as rrp:
                with tc.For_i(0, NB, 1, hint_engines=(mybir.EngineType.PE,),
                              staggered_reset=True) as j:
                    # open each bank's block group: bias + input drive
                    for mi in range(MC):
                        pxv = big[:, mi, 0:U * BL]
                        nc.tensor.matmul(pxv, lhsT=bahq[0:1, mi, :],
                                         rhs=ones[0:1, :],
                                         start=True, stop=False)
                        nc.tensor.matmul(pxv, lhsT=wx[:, mi, :],
                                         rhs=xTv[:, bass.ds(j, 1), :, :],
                                         start=False, stop=False,
                                         skip_group_check=True)

                    for th in range(U):
                        hf = th % 2          # first-updated half
                        hs = 1 - hf
                        s_r = (th - 1) % U
                        s_w = th

                        def quad(mh, kh, stop):
                            for mloc in range(MH):
                                mi = mh * MH + mloc
                                for kloc in range(KH):
                                    ki = kh * KH + kloc
                                    nc.tensor.matmul(
                                        big[:, mi, th * BL:(th + 1) * BL],
                                        lhsT=wsta[:, ki, mi, :],
                                        rhs=ring[:, s_r, ki, :],
                                        start=False,
                                        stop=(stop and ki == KC - 1),
                                        skip_group_check=True)

                        def update(mh, prev_ts, prev_tanh):
                            sl = slice(mh * MH, mh * MH + MH)
                            psv = big[:, sl, th * BL:(th + 1) * BL]
                            stt = nc.vector.scalar_tensor_tensor(
                                out=ah[:, sl, :], in0=ah[:, sl, :],
                                scalar=DECAY, in1=psv,
                                op0=AluOpType.mult, op1=AluOpType.add)
                            if prev_ts is not None:
                                _order_dep(stt, prev_ts)
                            rr = rrp.tile([128, MH, BL], F32, tag="rr")
                            ts = nc.vector.tensor_scalar_max(
                                rr, ah[:, sl, :], 0.0)
                            # two tanh ops per half: the first chunk pair
                            # publishes earlier, ungating the next stream's
                            # leading matmuls sooner
                            tk = prev_tanh
                            for p2 in range(2):
                                c0 = mh * MH + p2 * 2
                                t2 = nc.scalar.activation(
                                    ring[:, s_w, c0:c0 + 2, :],
                                    rr[:, p2 * 2:p2 * 2 + 2, :], AF.Tanh)
                                if tk is not None:
                                    _order_dep(t2, tk)
                                tk = t2
                            return ts, tk

                        last = (th == U - 1)
                        quad(hf, hs, False)            # q1: first m, early k
                        quad(hs, hs, False)            # q2: second m, early k
                        quad(hf, hf, last)             # q3: closes first m
                        ts1, tk1 = update(hf, None, None)
                        quad(hs, hf, last)             # q4: closes second m
                        update(hs, ts1, tk1)

                    # y readout from the PSUM bank tails (banks 4..7)
                    copies = []
                    for q, (a, b) in enumerate(YSPLIT):
                        fd = (b - a) * BL
                        ypv = big[0:NO, MH + q, 400:400 + fd]
                        for ki in range(KC):
                            nc.tensor.matmul(ypv, lhsT=wy[:, ki, :],
                                             rhs=ring[:, a:b, ki, :],
                                             start=(ki == 0),
                                             stop=(ki == KC - 1),
                                             skip_group_check=True)
                        copies.append((ypv, a, b, fd))
                    ybv = yb.rearrange("o n u b -> o n (u b)")
                    for ypv, a, b, fd in copies:
                        nc.vector.tensor_copy(
                            ybv[:, bass.ds(j, 1), a * BL:b * BL],
                            _bcast_ap(ypv, [1, fd], [0, 1]))

            # ---- Post: add by, write y out ----
            ybf = yb.rearrange("o n u b -> o (n u b)")
            nc.scalar.activation(ybf, ybf, AF.Identity, bias=byv[:, 0:1],
                                 scale=1.0)
            nc.sync.dma_start(y_d[:], yb.rearrange("o n u b -> o (n u) b"))

    nc.compile()
    return nc


_NC_CACHE = {}


def _get_nc():
    if "nc" not in _NC_CACHE:
        _NC_CACHE["nc"] = _build_nc()
    return _NC_CACHE["nc"]


def prepare_in_maps(x, Wahx, Wahh, Wyh, bah, by, ah0, mask):
    bf16 = ml_dtypes.bfloat16
    x = np.asarray(x, np.float32)
    Wahx = np.asarray(Wahx, np.float32)
    Wahh = np.asarray(Wahh, np.float32)
    Wyh = np.asarray(Wyh, np.float32)
    bah = np.asarray(bah, np.float32)
    by = np.asarray(by, np.float32)
    ah0 = np.asarray(ah0, np.float32)
    mask = np.asarray(mask, np.float32)

    weff = np.abs(Wahh) * mask                       # [m, k]
    wq = (DT * weff).reshape(MC, 128, KC, 128)       # [mi, mm, ki, kk]
    wq_l = np.ascontiguousarray(wq.transpose(3, 2, 0, 1)).astype(bf16)
    wx_l = np.ascontiguousarray(
        (DT * Wahx).T.reshape(NI, MC, 128)).astype(bf16)
    wy_l = np.ascontiguousarray(
        Wyh.T.reshape(KC, 128, NO).transpose(1, 0, 2)).astype(bf16)
    bahq_l = np.ascontiguousarray(
        (DT * bah).reshape(1, MC, 128)).astype(bf16)
    ones_l = np.ones((1, U * BL), dtype=bf16)
    ah0_l = np.ascontiguousarray(ah0.reshape(MC, 128).T, dtype=np.float32)
    by_l = np.ascontiguousarray(by.reshape(NO, 1), dtype=np.float32)

    x16 = x.reshape(B, T * NI).astype(bf16)
    in_maps = []
    for c in range(NCORES):
        xc = np.ascontiguousarray(
            x16[c * BL:(c + 1) * BL].reshape(BL * T, NI))
        in_maps.append(dict(x=xc, wq=wq_l, wx=wx_l, wy=wy_l, bahq=bahq_l,
                            ones=ones_l, ah0=ah0_l, by=by_l))
    return in_maps


def kernel(x, Wahx, Wahh, Wyh, bah, by, ah0, mask):
    global LAST_RESULTS
    in_maps = prepare_in_maps(x, Wahx, Wahh, Wyh, bah, by, ah0, mask)
    nc = _get_nc()
    res = run_bass_kernel_spmd(nc, in_maps, core_ids=list(range(NCORES)),
                               trace=TRACE)
    LAST_RESULTS = res

    out = np.empty((B, T, NO), np.float32)
    for c in range(NCORES):
        yc = np.asarray(res.results[c]["y"], np.float32)   # [NO, T, BL]
        out[c * BL:(c + 1) * BL] = yc.transpose(2, 1, 0)
    return out
